# revision 66
# baseline (speedup 1.0000x reference)
"""TRN2 Bass kernel for GQA MultiHeadAttention (B=2, S=2048, D=2048, 16 q-heads,
4 kv-heads, d_k=128) with QK-RMSNorm + interleaved RoPE + causal softmax + out-proj.

Sharding: 8 cores = (batch b in {0,1}) x (kv-head group g in {0..3}).
Each core computes its 4 q-heads' attention for its batch and a partial
out-projection y.T = Wo_g @ attn_out_g.T  [2048(e) x 2048(s)].
Host sums the 4 partials per batch and transposes.

Device layouts (all "head-dim on partitions", so no on-device transposes):
  xT   [d=16x128, s]        (moving operand of all projections)
  qT/kT [c=128, s]          RoPE'd + normalized, bf16
  V    [s-in-block=128, 16 blocks, c=128]
  scores ST [j=128, i<=512] via matmul(lhsT=kT-block, rhs=qT-tile)
  P = exp(ST * c0 * rrk[j]) masked; AO.T [c, i] = sum_j V.T P
No softmax max-subtraction: RMSNorm bounds |score| <= sqrt(128), exp is safe.
RoPE pair-interleave is folded into a host-side row permutation of Wq/Wk
(dot products are invariant; V/Wo untouched).
"""
import sys
import numpy as np
import ml_dtypes

sys.path.insert(0, "/opt/trn_rl_repo")

import concourse.bass as bass  # noqa: E402
import concourse.tile as tile  # noqa: E402
from concourse import mybir  # noqa: E402
from concourse.bass_utils import run_bass_kernel_spmd  # noqa: E402

F32 = mybir.dt.float32
F32R = mybir.dt.float32r
BF16 = mybir.dt.bfloat16
AF = mybir.ActivationFunctionType

P = 128
S = 2048
D = 2048
DK = 128
NH_LOC = 4          # q heads per core
NC_CHUNKS = D // P  # 16 contraction chunks
N_STILE = 4         # s-tiles of 512
STILE = 512
NJB = S // P        # 16 j/s blocks of 128
EPS = 1e-8
C0 = 1.0 / np.sqrt(DK)

_BF = ml_dtypes.bfloat16


_NO_SPLIT_OPCODES = {"UnconditionalBranch", "Call", "RegisterMove", "EventSemaphore"}
_WAIT_LIMIT = {}  # hw instruction structs take a single sync wait


def _split_excess_waits(nc):
    """Walrus codegen allows only 1-2 sync waits per instruction struct; Tile
    can emit more. Move excess waits onto same-engine NoOps inserted before."""
    import bass_rust
    counter = [0]
    for fn in nc.m.functions:
        for blk in fn.blocks:
            out = []
            changed = False
            for inst in blk.instructions:
                si = inst.sync_info
                limit = _WAIT_LIMIT.get(inst.opcode, 1)
                if (si is not None and len(si.on_wait) > limit
                        and inst.opcode not in _NO_SPLIT_OPCODES):
                    waits = list(si.on_wait)
                    for w in waits[:-limit]:
                        counter[0] += 1
                        nop = bass_rust.InstNoOp(
                            name=f"I-wsplit-{counter[0]}", ins=[], outs=[])
                        nop.engine = inst.engine
                        nop.sync_info = mybir.SyncInfo(on_wait=[w], on_update=[])
                        out.append(nop)
                    inst.sync_info = mybir.SyncInfo(
                        on_wait=waits[-limit:], on_update=list(si.on_update))
                    changed = True
                out.append(inst)
            if changed:
                blk.instructions = out
    return counter[0]


def _build_program():
    nc = bass.Bass()

    ext = {}

    def inp(name, shape, dt):
        ext[name] = nc.declare_dram_parameter(name, list(shape), dt, isOutput=False)
        return ext[name]

    xt4 = inp("xt4", (N_STILE, NC_CHUNKS, P, STILE), BF16)
    wq = inp("wq", (NC_CHUNKS, P, NH_LOC * DK), BF16)
    wk = inp("wk", (NC_CHUNKS, P, DK), BF16)
    wv = inp("wv", (NC_CHUNKS, P, DK), BF16)
    wo = inp("wo", (NH_LOC, P, D), BF16)
    winvq = inp("winvq", (P, 1), BF16)
    winvk = inp("winvk", (P, 1), BF16)
    taba = inp("taba", (P, S), BF16)  # [cos; cos]
    tabb = inp("tabb", (P, S), BF16)  # [-sin; sin]
    maskt = inp("maskt", (P, NH_LOC, STILE), BF16)
    bq = inp("bq", (P, NH_LOC), F32)
    bk = inp("bk", (P, 1), F32)
    eyer = inp("eyer", (P, P), BF16)
    onesf = inp("onesf", (P, P), BF16)
    yT = nc.declare_dram_parameter("yT", [D, S], BF16, isOutput=True)

    from contextlib import ExitStack

    with tile.TileContext(nc) as tc, ExitStack() as top:
        const = top.enter_context(tc.tile_pool(name="const", bufs=1))

        wq_sb = const.tile([P, NC_CHUNKS, NH_LOC * DK], BF16, tag="wq")
        wk_sb = const.tile([P, NC_CHUNKS, DK], BF16, tag="wk")
        wv_sb = const.tile([P, NC_CHUNKS, DK], BF16, tag="wv")
        winvq_sb = const.tile([P, 1], BF16, tag="winvq")
        winvk_sb = const.tile([P, 1], BF16, tag="winvk")
        taba_sb = const.tile([P, S], BF16, tag="taba")
        tabb_sb = const.tile([P, S], BF16, tag="tabb")
        mask_sb = const.tile([P, NH_LOC, STILE], BF16, tag="mask")
        bq_sb = const.tile([P, NH_LOC], F32, tag="bq")
        bk_sb = const.tile([P, 1], F32, tag="bk")
        wo_sb = const.tile([P, NH_LOC, D], BF16, tag="wo")

        eye_sb = const.tile([P, P], BF16, tag="eye")
        nc.sync.dma_start(eye_sb[:], eyer[:])
        onesf_sb = const.tile([P, P], BF16, tag="onesf")
        nc.sync.dma_start(onesf_sb[:], onesf[:])
        onesc_b = const.tile([P, 1], BF16, tag="onesc")
        nc.vector.memset(onesc_b[:], 1.0)
        eps_sb = const.tile([P, 1], F32, tag="eps")
        nc.vector.memset(eps_sb[:], EPS)
        lnc0_sb = const.tile([P, 1], F32, tag="lnc0")
        nc.vector.memset(lnc0_sb[:], float(np.log(C0)))

        # persistent activation tensors
        qhat = const.tile([P, NH_LOC, S], BF16, tag="qhat")   # [c, h, s]
        khat = const.tile([P, S], BF16, tag="khat")           # [c, s]
        vsb = const.tile([P, NJB, DK], BF16, tag="v")         # [s%128, block, c]
        aon = const.tile([P, NH_LOC, S], BF16, tag="aon")     # [c, h, i]
        scale_k = const.tile([P, NJB], F32, tag="scale_k")    # c0 * rrk, [j%128, block]

        # ------- Phase 1: projections + RMS + RoPE, fused per (output, s-tile) -------
        with ExitStack() as ph1:
            xp = ph1.enter_context(tc.tile_pool(name="xp", bufs=2))
            t1p = ph1.enter_context(tc.tile_pool(name="t1p", bufs=4))
            rp = ph1.enter_context(tc.tile_pool(name="rp", bufs=3))
            ps1 = ph1.enter_context(tc.tile_pool(name="ps1", bufs=3, space="PSUM"))
            pskl = ph1.enter_context(tc.tile_pool(name="pskl", bufs=1, space="PSUM"))
            psb = ph1.enter_context(tc.tile_pool(name="psb", bufs=2, space="PSUM"))

            ps_kl = pskl.tile([P, NJB], F32, tag="pskl")

            # startup-critical loads first: q weights on SP, first x tile on
            # the ACT queue — parallel dispatch halves time-to-first-matmul
            xt0 = xp.tile([P, NC_CHUNKS, STILE], BF16, tag="xt", name="xt0")
            nc.sync.dma_start(wq_sb[:, 0:1, :], wq[0:1].rearrange("c p m -> p c m"))
            nc.scalar.dma_start(xt0[:, 0:1, :], xt4[0, 0:1].rearrange("c p s -> p c s"))
            for g4 in range(0, NC_CHUNKS, 4):
                sl4 = slice(max(g4, 1), g4 + 4)
                nc.sync.dma_start(wq_sb[:, sl4, :], wq[sl4].rearrange("c p m -> p c m"))
                nc.sync.dma_start(xt0[:, sl4, :],
                                  xt4[0, sl4].rearrange("c p s -> p c s"))
            nc.sync.dma_start(bq_sb[:], bq[:])
            nc.sync.dma_start(winvq_sb[:], winvq[:])
            nc.sync.dma_start(taba_sb[:], taba[:])
            nc.sync.dma_start(tabb_sb[:], tabb[:])
            for g4 in range(0, NC_CHUNKS, 4):
                sl4 = slice(g4, g4 + 4)
                nc.sync.dma_start(wk_sb[:, sl4, :], wk[sl4].rearrange("c p m -> p c m"))
                nc.sync.dma_start(wv_sb[:, sl4, :], wv[sl4].rearrange("c p m -> p c m"))
            nc.sync.dma_start(winvk_sb[:], winvk[:])
            nc.sync.dma_start(bk_sb[:], bk[:])
            # mask is needed by the very first phase-2 tile; load it early
            nc.sync.dma_start(mask_sb[:], maskt[:])

            # deferred q-rms apply: pb broadcast matmul + qhat mul emitted one
            # output later, so the Ln/Exp latency hides under the next
            # output's projection matmuls instead of stalling PE in-order
            pend_q = [None]

            def flush_q():
                if pend_q[0] is None:
                    return
                oi, rt, mskq, pb, ssl_p = pend_q[0]
                pend_q[0] = None
                nc.tensor.matmul(pb[:], onesf_sb[:], mskq[:],
                                 start=True, stop=True)
                nc.vector.tensor_mul(qhat[:, oi, ssl_p], rt[:], pb[:])

            for st in range(N_STILE):
                if st == 0:
                    xt = xt0
                else:
                    xt = xp.tile([P, NC_CHUNKS, STILE], BF16, tag="xt")
                    for g4 in range(0, NC_CHUNKS, 4):
                        sl4 = slice(g4, g4 + 4)
                        nc.sync.dma_start(xt[:, sl4, :],
                                          xt4[st, sl4].rearrange("c p s -> p c s"))
                ssl = bass.ts(st, STILE)

                # q heads first, then v, then k (k last: its psum banks are
                # freed by the fast ACT reader right before phase 2 reuses them)
                for oi in list(range(NH_LOC)) + ["v", "k"]:
                    if oi == "v":
                        # v: output [s-block=128, c=128], 4 s-blocks per s-tile
                        ptv = ps1.tile([P, STILE], F32, tag="proj", name="ptv")
                        for sb in range(4):
                            for ch in range(NC_CHUNKS):
                                nc.tensor.matmul(ptv[:, bass.ts(sb, DK)],
                                                 xt[:, ch, bass.ts(sb, P)], wv_sb[:, ch, :],
                                                 start=(ch == 0), stop=(ch == NC_CHUNKS - 1))
                        flush_q()
                        for sb in range(4):
                            nc.vector.tensor_copy(vsb[:, st * 4 + sb, :], ptv[:, bass.ts(sb, DK)])
                        continue
                    is_q = oi != "k"
                    pt = ps1.tile([P, STILE], F32, tag="proj")
                    for ch in range(NC_CHUNKS):
                        lw = wq_sb[:, ch, bass.ts(oi, DK)] if is_q else wk_sb[:, ch, :]
                        nc.tensor.matmul(pt[:], lw, xt[:, ch, :],
                                         start=(ch == 0), stop=(ch == NC_CHUNKS - 1))
                    flush_q()
                    bias_ap = bq_sb[:, oi : oi + 1] if is_q else bk_sb[:, 0:1]
                    qf = t1p.tile([P, STILE], BF16, tag="qf")
                    nc.scalar.activation(qf[:], pt[:], AF.Identity, bias=bias_ap)

                    # sumsq for rms (q: [1,s] orientation; k: transposed [s,1])
                    sq = t1p.tile([P, STILE], BF16, tag="sq")
                    nc.vector.tensor_mul(sq[:], qf[:], qf[:])
                    if is_q:
                        # sumsq transposed: 4 single-col matmuls -> [s%128, 4],
                        # parked in the first 4 columns of the pb bank
                        pb = psb.tile([P, STILE], F32, tag="rqb")
                        ltq = pb[:, 0:4]
                        for sb in range(4):
                            nc.tensor.matmul(ltq[:, sb : sb + 1],
                                             sq[:, bass.ts(sb, P)], winvq_sb[:],
                                             start=True, stop=True)
                        # rrq = 1/sqrt(mean+eps) = exp(-0.5*ln(v/DK + eps))
                        lnq = t1p.tile([P, 4], F32, tag="lnv")
                        nc.scalar.activation(lnq[:], ltq[:], AF.Ln,
                                             bias=eps_sb[:], scale=1.0 / DK)
                        rrq2 = t1p.tile([P, 4], F32, tag="rrq")
                        nc.scalar.activation(rrq2[:], lnq[:], AF.Exp, scale=-0.5)
                        mskq = rp.tile([P, STILE], BF16, tag="mskq")
                        for c2 in range(4):
                            nc.vector.tensor_scalar_mul(mskq[:, bass.ts(c2, P)],
                                                        eye_sb[:],
                                                        rrq2[:, c2 : c2 + 1])
                    else:
                        for sb in range(4):
                            nc.tensor.matmul(ps_kl[:, st * 4 + sb : st * 4 + sb + 1],
                                             sq[:, bass.ts(sb, P)], winvk_sb[:],
                                             start=True, stop=True)

                    # RoPE on de-interleaved halves:
                    #   rt = qf*[cos;cos] + swap_halves(qf)*[-sin;sin]
                    # (swap done by cross-partition-offset reads of qf)
                    ta = rp.tile([P, STILE], BF16, tag="ta")
                    tb = rp.tile([P, STILE], BF16, tag="tb")
                    nc.vector.tensor_mul(ta[:], qf[:], taba_sb[:, ssl])
                    nc.vector.tensor_mul(tb[0:64, :], qf[64:P, :], tabb_sb[64:P, ssl])
                    nc.vector.tensor_mul(tb[64:P, :], qf[0:64, :], tabb_sb[0:64, ssl])
                    if is_q:
                        rt = rp.tile([P, STILE], BF16, tag="rope")
                        nc.vector.tensor_add(rt[:], ta[:], tb[:])
                        pend_q[0] = (oi, rt, mskq, pb, ssl)
                    else:
                        nc.vector.tensor_add(khat[:, ssl], ta[:], tb[:])

                # k-side scale for this s-tile: c0 / rms_k as [j%128, block]
                # = exp(-0.5*ln(mean+eps) + ln(c0)), single-table Ln/Exp
                ksl4 = bass.ts(st, 4)
                lnk = t1p.tile([P, 4], F32, tag="lnk")
                nc.scalar.activation(lnk[:], ps_kl[:, ksl4], AF.Ln,
                                     bias=eps_sb[:], scale=1.0 / DK)
                nc.scalar.activation(scale_k[:, ksl4], lnk[:], AF.Exp,
                                     scale=-0.5, bias=lnc0_sb[:])




        # ---------------- Phase 2: attention ----------------
        nc.sync.dma_start(wo_sb[:], wo.rearrange("f p e -> p f e"))

        yp = top.enter_context(tc.tile_pool(name="yp", bufs=6))
        yT_v = yT.rearrange("(eb p) s -> eb p s", p=P)
        y_sbs = {}

        def outproj_group(eb, st, pspool):
            """One (eb, st) out-projection accumulation + PSUM drain."""
            if eb not in y_sbs:
                y_sbs[eb] = yp.tile([P, S], BF16, tag="ysb", name=f"ysb{eb}")
            y_sb = y_sbs[eb]
            yps = pspool.tile([P, STILE], F32, tag="st", name=f"yps{eb}_{st}")
            for fc in range(NH_LOC):
                nc.tensor.matmul(yps[:], wo_sb[:, fc, bass.ts(eb, P)],
                                 aon[:, fc, bass.ts(st, STILE)],
                                 start=(fc == 0), stop=(fc == NH_LOC - 1))
            if (eb * N_STILE + st) % 2 == 0:
                nc.scalar.copy(y_sb[:, bass.ts(st, STILE)], yps[:])
            else:
                nc.vector.tensor_copy(y_sb[:, bass.ts(st, STILE)], yps[:])
            if eb >= NJB - 2:
                # stream the last rows out per-stile to shorten the tail
                nc.sync.dma_start(yT_v[eb][:, bass.ts(st, STILE)],
                                  y_sb[:, bass.ts(st, STILE)])
            elif st == N_STILE - 1:
                nc.sync.dma_start(yT_v[eb], y_sb[:])

        with ExitStack() as ph2:
            pp = ph2.enter_context(tc.tile_pool(name="pp", bufs=8))
            sap = ph2.enter_context(tc.tile_pool(name="sap", bufs=2))
            lp = ph2.enter_context(tc.tile_pool(name="lp", bufs=2))
            rlp = ph2.enter_context(tc.tile_pool(name="rlp", bufs=2))
            psst = ph2.enter_context(tc.tile_pool(name="psst", bufs=4, space="PSUM"))
            psao = ph2.enter_context(tc.tile_pool(name="psao", bufs=3, space="PSUM"))
            psrl = ph2.enter_context(tc.tile_pool(name="psrl", bufs=1, space="PSUM"))

            def normalize(pend):
                """Emit the deferred softmax-normalize for one (h, it) tile.
                Runs one tile late so the DVE->ACT->DVE chain never blocks
                PE's in-order queue."""
                h, isl, sacc, ao_ps = pend
                # l[i] = sum_j sacc: 4 single-column matmuls -> [i%128, 4].
                # lt borrows the first 4 columns of the rlb bank (disjoint
                # lifetime: Ln reads lt before the rlb matmul overwrites it).
                rlb = psrl.tile([P, STILE], F32, tag="rlb")
                lt = rlb[:, 0:4]
                for c2 in range(4):
                    nc.tensor.matmul(lt[:, c2 : c2 + 1],
                                     sacc[:, bass.ts(c2, P)], onesc_b[:],
                                     start=True, stop=True)
                # 1/l = exp(-ln(l)) on ACT, cheap at [128, 4]
                lnl = lp.tile([P, 4], F32, tag="lnl")
                nc.scalar.activation(lnl[:], lt[:], AF.Ln)
                rl2 = lp.tile([P, 4], F32, tag="rl2")
                nc.scalar.activation(rl2[:], lnl[:], AF.Exp, scale=-1.0)
                # transpose+broadcast 1/l to [c, i]: mask into eye columns,
                # then ones.T @ masked sums the single nonzero per column
                mskd = rlp.tile([P, STILE], BF16, tag="mskd")
                for c2 in range(4):
                    nc.vector.tensor_scalar_mul(mskd[:, bass.ts(c2, P)],
                                                eye_sb[:], rl2[:, c2 : c2 + 1])
                nc.tensor.matmul(rlb[:], onesf_sb[:], mskd[:],
                                 start=True, stop=True)
                rlb_sb = rlp.tile([P, STILE], F32, tag="rlbs")
                nc.vector.tensor_copy(rlb_sb[:], rlb[:])
                nc.vector.tensor_mul(aon[:, h, isl], ao_ps[:], rlb_sb[:])

            pending = None
            for h in range(NH_LOC):
                for it in range(N_STILE):
                    isl = bass.ts(it, STILE)
                    njb = 4 * it + 4
                    ao_ps = psao.tile([P, STILE], F32, tag="ao")
                    # P-tile running sum over j-blocks (for the softmax denom)
                    sacc = sap.tile([P, STILE], BF16, tag="sacc")
                    # ascending jb: group head (jb=0) is always full-width, so
                    # later diagonal blocks can write narrowed column ranges.
                    for idx, jb in enumerate(range(njb)):
                        t = jb - 4 * it
                        lo = P * t if t > 0 else 0  # masked-out prefix columns
                        csl = slice(lo, STILE)
                        i0 = it * STILE + lo
                        st_ps = psst.tile([P, STILE], F32, tag="st")
                        nc.tensor.matmul(st_ps[:, csl], khat[:, bass.ts(jb, P)],
                                         qhat[:, h, bass.ds(i0, STILE - lo)],
                                         start=True, stop=True)
                        pt = pp.tile([P, STILE], BF16, tag="p")
                        nc.scalar.activation(pt[:, csl], st_ps[:, csl], AF.Exp,
                                             scale=scale_k[:, jb : jb + 1])
                        if t >= 0:
                            nc.vector.tensor_mul(pt[:, csl], pt[:, csl],
                                                 mask_sb[:, t, csl])
                        nc.tensor.matmul(ao_ps[:, csl], vsb[:, jb, :], pt[:, csl],
                                         start=(idx == 0), stop=(idx == njb - 1))
                        if idx == 0:
                            nc.vector.tensor_copy(sacc[:], pt[:])
                        else:
                            nc.vector.tensor_add(sacc[:, csl], sacc[:, csl],
                                                 pt[:, csl])
                    if pending is not None:
                        normalize(pending)
                    pending = (h, isl, sacc, ao_ps)

            # first out-proj groups act as PE filler while the last tile's
            # normalize chain (DVE/ACT) completes
            for eb in range(6):
                for st in range(N_STILE - 1):
                    outproj_group(eb, st, psst)
            normalize(pending)
            for eb in range(6):
                outproj_group(eb, N_STILE - 1, psst)

        # ---------------- Phase 3: out-projection (rest) ----------------
        # bias bo is added host-side after the partial gather
        with ExitStack() as ph3:
            psy = ph3.enter_context(tc.tile_pool(name="psy", bufs=6, space="PSUM"))
            for eb in range(6, NJB):
                for st in range(N_STILE):
                    outproj_group(eb, st, psy)

    _split_excess_waits(nc)
    return nc


_PERM = np.concatenate([np.arange(0, DK, 2), np.arange(1, DK, 2)])  # de-interleave


def _prep_inputs(x, Wq, bq, Wk, bk, Wv, bv, Wo, bo, q_norm_w, k_norm_w):
    """Build the 8 per-core input maps. Core c -> (b = c // 4, g = c % 4)."""
    def bf(a):
        return np.ascontiguousarray(a).astype(_BF)

    wq_p = q_norm_w[_PERM].astype(np.float32)
    wk_p = k_norm_w[_PERM].astype(np.float32)
    with np.errstate(divide="ignore"):
        winvq = np.where(wq_p != 0, 1.0 / np.maximum(wq_p * wq_p, 1e-30), 0.0)
        winvk = np.where(wk_p != 0, 1.0 / np.maximum(wk_p * wk_p, 1e-30), 0.0)

    inv_freq = 1.0 / (10000.0 ** (np.arange(0, DK, 2, dtype=np.float32) / np.float32(DK)))
    freqs = np.arange(S, dtype=np.float32)[:, None] * inv_freq[None, :]
    cosT = np.cos(freqs).T.astype(np.float32)  # [64, S]
    sinT = np.sin(freqs).T.astype(np.float32)
    taba = bf(np.vstack([cosT, cosT]))             # [128, S]
    # tb is computed by cross-half reads (tb_lo = qf_hi*tabb_hi), so the
    # sign rows are pre-swapped: tabb[p] multiplies qf[(p+64)%128]
    tabb = bf(np.vstack([sinT, -sinT]))

    pj = np.arange(P)[:, None, None]
    tt = np.arange(NH_LOC)[None, :, None]
    fi = np.arange(STILE)[None, None, :]
    maskt = ((P * tt + pj) <= fi).astype(np.float32)  # [128, 4, 512]

    xt4_b = []
    for b in range(2):
        xt = x[b].T.astype(np.float32)  # [d, s]
        xt4_b.append(bf(xt.reshape(NC_CHUNKS, P, N_STILE, STILE).transpose(2, 0, 1, 3)))

    in_maps = []
    for core in range(8):
        b, g = divmod(core, NH_LOC)
        hsl = slice(g * NH_LOC * DK, (g + 1) * NH_LOC * DK)
        ksl = slice(g * DK, (g + 1) * DK)

        wq_blk = Wq[hsl].astype(np.float32).copy()  # [512, d]
        # per-head de-interleave permutation + fold q_norm_w
        wq_blk = wq_blk.reshape(NH_LOC, DK, D)[:, _PERM, :] * wq_p[None, :, None]
        wq_t = wq_blk.reshape(NH_LOC * DK, D).T.reshape(NC_CHUNKS, P, NH_LOC * DK)

        wk_blk = Wk[ksl].astype(np.float32)[_PERM, :] * wk_p[:, None]
        wk_t = wk_blk.T.reshape(NC_CHUNKS, P, DK)
        wv_t = Wv[ksl].astype(np.float32).T.reshape(NC_CHUNKS, P, DK)
        wo_t = Wo[:, hsl].astype(np.float32).T.reshape(NH_LOC, P, D)

        bq_blk = bq[hsl].astype(np.float32).reshape(NH_LOC, DK)[:, _PERM].T.copy()  # [128, 4]
        bk_blk = bk[ksl].astype(np.float32)[_PERM][:, None].copy()

        in_maps.append({
            "xt4": xt4_b[b],
            "wq": bf(wq_t), "wk": bf(wk_t), "wv": bf(wv_t), "wo": bf(wo_t),
            "winvq": bf(winvq[:, None]), "winvk": bf(winvk[:, None]),
            "taba": taba, "tabb": tabb,
            "maskt": bf(maskt),
            "bq": np.ascontiguousarray(bq_blk), "bk": bk_blk,
            "eyer": bf(np.eye(P, dtype=np.float32)),
            "onesf": bf(np.ones((P, P), np.float32)),
        })
    return in_maps


_CACHED = {}


def _get_program():
    if "nc" not in _CACHED:
        _CACHED["nc"] = _build_program()
    return _CACHED["nc"]


def kernel(x, Wq, bq, Wk, bk, Wv, bv, Wo, bo, q_norm_w, k_norm_w, _trace=False, _tmpdir=None):
    x = np.asarray(x, np.float32)
    args = [np.asarray(a, np.float32) for a in
            (Wq, bq, Wk, bk, Wv, bv, Wo, bo, q_norm_w, k_norm_w)]
    Wq, bq, Wk, bk, Wv, bv, Wo, bo, q_norm_w, k_norm_w = args

    nc = _get_program()
    in_maps = _prep_inputs(x, Wq, bq, Wk, bk, Wv, bv, Wo, bo, q_norm_w, k_norm_w)
    res = run_bass_kernel_spmd(nc, in_maps, list(range(8)), trace=_trace, tmpdir=_tmpdir)

    out = np.zeros((2, S, D), np.float32)
    for core in range(8):
        b = core // 4
        out[b] += res.results[core]["yT"].astype(np.float32).T
    out += bo[None, None, :]
    # v-bias enters only via softmax-weighted average (weights sum to 1):
    if np.any(bv):
        out += (np.repeat(bv.reshape(4, DK), 4, axis=0).reshape(D) @ Wo.T)[None, None, :]
    kernel._last_result = res
    return out



# revision 67
# speedup vs baseline: 1.0030x; 1.0030x over previous
"""TRN2 Bass kernel for GQA MultiHeadAttention (B=2, S=2048, D=2048, 16 q-heads,
4 kv-heads, d_k=128) with QK-RMSNorm + interleaved RoPE + causal softmax + out-proj.

Sharding: 8 cores = (batch b in {0,1}) x (kv-head group g in {0..3}).
Each core computes its 4 q-heads' attention for its batch and a partial
out-projection y.T = Wo_g @ attn_out_g.T  [2048(e) x 2048(s)].
Host sums the 4 partials per batch and transposes.

Device layouts (all "head-dim on partitions", so no on-device transposes):
  xT   [d=16x128, s]        (moving operand of all projections)
  qT/kT [c=128, s]          RoPE'd + normalized, bf16
  V    [s-in-block=128, 16 blocks, c=128]
  scores ST [j=128, i<=512] via matmul(lhsT=kT-block, rhs=qT-tile)
  P = exp(ST * c0 * rrk[j]) masked; AO.T [c, i] = sum_j V.T P
No softmax max-subtraction: RMSNorm bounds |score| <= sqrt(128), exp is safe.
RoPE pair-interleave is folded into a host-side row permutation of Wq/Wk
(dot products are invariant; V/Wo untouched).
"""
import sys
import numpy as np
import ml_dtypes

sys.path.insert(0, "/opt/trn_rl_repo")

import concourse.bass as bass  # noqa: E402
import concourse.tile as tile  # noqa: E402
from concourse import mybir  # noqa: E402
from concourse.bass_utils import run_bass_kernel_spmd  # noqa: E402

F32 = mybir.dt.float32
F32R = mybir.dt.float32r
BF16 = mybir.dt.bfloat16
AF = mybir.ActivationFunctionType

P = 128
S = 2048
D = 2048
DK = 128
NH_LOC = 4          # q heads per core
NC_CHUNKS = D // P  # 16 contraction chunks
N_STILE = 4         # s-tiles of 512
STILE = 512
NJB = S // P        # 16 j/s blocks of 128
EPS = 1e-8
C0 = 1.0 / np.sqrt(DK)

_BF = ml_dtypes.bfloat16


_NO_SPLIT_OPCODES = {"UnconditionalBranch", "Call", "RegisterMove", "EventSemaphore"}
_WAIT_LIMIT = {}  # hw instruction structs take a single sync wait


def _split_excess_waits(nc):
    """Walrus codegen allows only 1-2 sync waits per instruction struct; Tile
    can emit more. Move excess waits onto same-engine NoOps inserted before."""
    import bass_rust
    counter = [0]
    for fn in nc.m.functions:
        for blk in fn.blocks:
            out = []
            changed = False
            for inst in blk.instructions:
                si = inst.sync_info
                limit = _WAIT_LIMIT.get(inst.opcode, 1)
                if (si is not None and len(si.on_wait) > limit
                        and inst.opcode not in _NO_SPLIT_OPCODES):
                    waits = list(si.on_wait)
                    for w in waits[:-limit]:
                        counter[0] += 1
                        nop = bass_rust.InstNoOp(
                            name=f"I-wsplit-{counter[0]}", ins=[], outs=[])
                        nop.engine = inst.engine
                        nop.sync_info = mybir.SyncInfo(on_wait=[w], on_update=[])
                        out.append(nop)
                    inst.sync_info = mybir.SyncInfo(
                        on_wait=waits[-limit:], on_update=list(si.on_update))
                    changed = True
                out.append(inst)
            if changed:
                blk.instructions = out
    return counter[0]


def _build_program():
    nc = bass.Bass()

    ext = {}

    def inp(name, shape, dt):
        ext[name] = nc.declare_dram_parameter(name, list(shape), dt, isOutput=False)
        return ext[name]

    xt4 = inp("xt4", (N_STILE, NC_CHUNKS, P, STILE), BF16)
    wq = inp("wq", (NC_CHUNKS, P, NH_LOC * DK), BF16)
    wk = inp("wk", (NC_CHUNKS, P, DK), BF16)
    wv = inp("wv", (NC_CHUNKS, P, DK), BF16)
    wo = inp("wo", (NH_LOC, P, D), BF16)
    winvq = inp("winvq", (P, 1), BF16)
    winvk = inp("winvk", (P, 1), BF16)
    taba = inp("taba", (P, S), BF16)  # [cos; cos]
    tabb = inp("tabb", (P, S), BF16)  # [-sin; sin]
    maskt = inp("maskt", (P, NH_LOC, STILE), BF16)
    bq = inp("bq", (P, NH_LOC), F32)
    bk = inp("bk", (P, 1), F32)
    eyer = inp("eyer", (P, P), BF16)
    onesf = inp("onesf", (P, P), BF16)
    yT = nc.declare_dram_parameter("yT", [D, S], BF16, isOutput=True)

    from contextlib import ExitStack

    with tile.TileContext(nc) as tc, ExitStack() as top:
        const = top.enter_context(tc.tile_pool(name="const", bufs=1))

        wq_sb = const.tile([P, NC_CHUNKS, NH_LOC * DK], BF16, tag="wq")
        wk_sb = const.tile([P, NC_CHUNKS, DK], BF16, tag="wk")
        wv_sb = const.tile([P, NC_CHUNKS, DK], BF16, tag="wv")
        winvq_sb = const.tile([P, 1], BF16, tag="winvq")
        winvk_sb = const.tile([P, 1], BF16, tag="winvk")
        taba_sb = const.tile([P, S], BF16, tag="taba")
        tabb_sb = const.tile([P, S], BF16, tag="tabb")
        mask_sb = const.tile([P, NH_LOC, STILE], BF16, tag="mask")
        bq_sb = const.tile([P, NH_LOC], F32, tag="bq")
        bk_sb = const.tile([P, 1], F32, tag="bk")
        wo_sb = const.tile([P, NH_LOC, D], BF16, tag="wo")

        eye_sb = const.tile([P, P], BF16, tag="eye")
        nc.sync.dma_start(eye_sb[:], eyer[:])
        onesf_sb = const.tile([P, P], BF16, tag="onesf")
        nc.sync.dma_start(onesf_sb[:], onesf[:])
        onesc_b = const.tile([P, 1], BF16, tag="onesc")
        nc.vector.memset(onesc_b[:], 1.0)
        eps_sb = const.tile([P, 1], F32, tag="eps")
        nc.vector.memset(eps_sb[:], EPS)
        lnc0_sb = const.tile([P, 1], F32, tag="lnc0")
        nc.vector.memset(lnc0_sb[:], float(np.log(C0)))

        # persistent activation tensors
        qhat = const.tile([P, NH_LOC, S], BF16, tag="qhat")   # [c, h, s]
        khat = const.tile([P, S], BF16, tag="khat")           # [c, s]
        vsb = const.tile([P, NJB, DK], BF16, tag="v")         # [s%128, block, c]
        aon = const.tile([P, NH_LOC, S], BF16, tag="aon")     # [c, h, i]
        scale_k = const.tile([P, NJB], F32, tag="scale_k")    # c0 * rrk, [j%128, block]

        # ------- Phase 1: projections + RMS + RoPE, fused per (output, s-tile) -------
        with ExitStack() as ph1:
            xp = ph1.enter_context(tc.tile_pool(name="xp", bufs=2))
            t1p = ph1.enter_context(tc.tile_pool(name="t1p", bufs=4))
            rp = ph1.enter_context(tc.tile_pool(name="rp", bufs=3))
            ps1 = ph1.enter_context(tc.tile_pool(name="ps1", bufs=3, space="PSUM"))
            pskl = ph1.enter_context(tc.tile_pool(name="pskl", bufs=1, space="PSUM"))
            psb = ph1.enter_context(tc.tile_pool(name="psb", bufs=2, space="PSUM"))

            ps_kl = pskl.tile([P, NJB], F32, tag="pskl")

            # startup-critical loads first: q weights on SP, first x tile on
            # the ACT queue — parallel dispatch halves time-to-first-matmul
            xt0 = xp.tile([P, NC_CHUNKS, STILE], BF16, tag="xt", name="xt0")
            nc.sync.dma_start(wq_sb[:, 0:1, :], wq[0:1].rearrange("c p m -> p c m"))
            nc.scalar.dma_start(xt0[:, 0:1, :], xt4[0, 0:1].rearrange("c p s -> p c s"))
            for g4 in range(0, NC_CHUNKS, 4):
                sl4 = slice(max(g4, 1), g4 + 4)
                nc.sync.dma_start(wq_sb[:, sl4, :], wq[sl4].rearrange("c p m -> p c m"))
                nc.sync.dma_start(xt0[:, sl4, :],
                                  xt4[0, sl4].rearrange("c p s -> p c s"))
            nc.sync.dma_start(bq_sb[:], bq[:])
            nc.sync.dma_start(winvq_sb[:], winvq[:])
            nc.sync.dma_start(taba_sb[:], taba[:])
            nc.sync.dma_start(tabb_sb[:], tabb[:])
            for g4 in range(0, NC_CHUNKS, 4):
                sl4 = slice(g4, g4 + 4)
                nc.sync.dma_start(wk_sb[:, sl4, :], wk[sl4].rearrange("c p m -> p c m"))
                nc.sync.dma_start(wv_sb[:, sl4, :], wv[sl4].rearrange("c p m -> p c m"))
            nc.sync.dma_start(winvk_sb[:], winvk[:])
            nc.sync.dma_start(bk_sb[:], bk[:])
            # mask is needed by the very first phase-2 tile; load it early
            nc.sync.dma_start(mask_sb[:], maskt[:])

            # deferred q-rms apply: pb broadcast matmul + qhat mul emitted one
            # output later, so the Ln/Exp latency hides under the next
            # output's projection matmuls instead of stalling PE in-order
            pend_q = [None]

            def flush_q():
                if pend_q[0] is None:
                    return
                oi, rt, mskq, pb, ssl_p = pend_q[0]
                pend_q[0] = None
                nc.tensor.matmul(pb[:], onesf_sb[:], mskq[:],
                                 start=True, stop=True)
                nc.vector.tensor_mul(qhat[:, oi, ssl_p], rt[:], pb[:])

            for st in range(N_STILE):
                if st == 0:
                    xt = xt0
                else:
                    xt = xp.tile([P, NC_CHUNKS, STILE], BF16, tag="xt")
                    for g4 in range(0, NC_CHUNKS, 4):
                        sl4 = slice(g4, g4 + 4)
                        nc.sync.dma_start(xt[:, sl4, :],
                                          xt4[st, sl4].rearrange("c p s -> p c s"))
                ssl = bass.ts(st, STILE)

                # q heads first, then v, then k (k last: its psum banks are
                # freed by the fast ACT reader right before phase 2 reuses them)
                for oi in list(range(NH_LOC)) + ["v", "k"]:
                    if oi == "v":
                        # v: output [s-block=128, c=128], 4 s-blocks per s-tile
                        ptv = ps1.tile([P, STILE], F32, tag="proj", name="ptv")
                        for sb in range(4):
                            for ch in range(NC_CHUNKS):
                                nc.tensor.matmul(ptv[:, bass.ts(sb, DK)],
                                                 xt[:, ch, bass.ts(sb, P)], wv_sb[:, ch, :],
                                                 start=(ch == 0), stop=(ch == NC_CHUNKS - 1))
                        flush_q()
                        for sb in range(4):
                            nc.vector.tensor_copy(vsb[:, st * 4 + sb, :], ptv[:, bass.ts(sb, DK)])
                        continue
                    is_q = oi != "k"
                    pt = ps1.tile([P, STILE], F32, tag="proj")
                    for ch in range(NC_CHUNKS):
                        lw = wq_sb[:, ch, bass.ts(oi, DK)] if is_q else wk_sb[:, ch, :]
                        nc.tensor.matmul(pt[:], lw, xt[:, ch, :],
                                         start=(ch == 0), stop=(ch == NC_CHUNKS - 1))
                    flush_q()
                    bias_ap = bq_sb[:, oi : oi + 1] if is_q else bk_sb[:, 0:1]
                    qf = t1p.tile([P, STILE], BF16, tag="qf")
                    nc.scalar.activation(qf[:], pt[:], AF.Identity, bias=bias_ap)

                    # sumsq for rms (q: [1,s] orientation; k: transposed [s,1])
                    sq = t1p.tile([P, STILE], BF16, tag="sq")
                    nc.vector.tensor_mul(sq[:], qf[:], qf[:])
                    if is_q:
                        # sumsq transposed: 4 single-col matmuls -> [s%128, 4],
                        # parked in the first 4 columns of the pb bank
                        pb = psb.tile([P, STILE], F32, tag="rqb")
                        ltq = pb[:, 0:4]
                        for sb in range(4):
                            nc.tensor.matmul(ltq[:, sb : sb + 1],
                                             sq[:, bass.ts(sb, P)], winvq_sb[:],
                                             start=True, stop=True)
                        # rrq = 1/sqrt(mean+eps) = exp(-0.5*ln(v/DK + eps))
                        lnq = t1p.tile([P, 4], F32, tag="lnv")
                        nc.scalar.activation(lnq[:], ltq[:], AF.Ln,
                                             bias=eps_sb[:], scale=1.0 / DK)
                        rrq2 = t1p.tile([P, 4], F32, tag="rrq")
                        nc.scalar.activation(rrq2[:], lnq[:], AF.Exp, scale=-0.5)
                        mskq = rp.tile([P, STILE], BF16, tag="mskq")
                        for c2 in range(4):
                            nc.vector.tensor_scalar_mul(mskq[:, bass.ts(c2, P)],
                                                        eye_sb[:],
                                                        rrq2[:, c2 : c2 + 1])
                    else:
                        for sb in range(4):
                            nc.tensor.matmul(ps_kl[:, st * 4 + sb : st * 4 + sb + 1],
                                             sq[:, bass.ts(sb, P)], winvk_sb[:],
                                             start=True, stop=True)

                    # RoPE on de-interleaved halves:
                    #   rt = qf*[cos;cos] + swap_halves(qf)*[-sin;sin]
                    # (swap done by cross-partition-offset reads of qf)
                    ta = rp.tile([P, STILE], BF16, tag="ta")
                    tb = rp.tile([P, STILE], BF16, tag="tb")
                    nc.vector.tensor_mul(ta[:], qf[:], taba_sb[:, ssl])
                    nc.vector.tensor_mul(tb[0:64, :], qf[64:P, :], tabb_sb[64:P, ssl])
                    nc.vector.tensor_mul(tb[64:P, :], qf[0:64, :], tabb_sb[0:64, ssl])
                    if is_q:
                        rt = rp.tile([P, STILE], BF16, tag="rope")
                        nc.vector.tensor_add(rt[:], ta[:], tb[:])
                        pend_q[0] = (oi, rt, mskq, pb, ssl)
                    else:
                        nc.vector.tensor_add(khat[:, ssl], ta[:], tb[:])

                # k-side scale for this s-tile: c0 / rms_k as [j%128, block]
                # = exp(-0.5*ln(mean+eps) + ln(c0)), single-table Ln/Exp
                ksl4 = bass.ts(st, 4)
                lnk = t1p.tile([P, 4], F32, tag="lnk")
                nc.scalar.activation(lnk[:], ps_kl[:, ksl4], AF.Ln,
                                     bias=eps_sb[:], scale=1.0 / DK)
                nc.scalar.activation(scale_k[:, ksl4], lnk[:], AF.Exp,
                                     scale=-0.5, bias=lnc0_sb[:])




        # ---------------- Phase 2: attention ----------------
        nc.sync.dma_start(wo_sb[:], wo.rearrange("f p e -> p f e"))

        yp = top.enter_context(tc.tile_pool(name="yp", bufs=6))
        yT_v = yT.rearrange("(eb p) s -> eb p s", p=P)
        y_sbs = {}

        def outproj_group(eb, st, pspool):
            """One (eb, st) out-projection accumulation + PSUM drain."""
            if eb not in y_sbs:
                y_sbs[eb] = yp.tile([P, S], BF16, tag="ysb", name=f"ysb{eb}")
            y_sb = y_sbs[eb]
            yps = pspool.tile([P, STILE], F32, tag="st", name=f"yps{eb}_{st}")
            for fc in range(NH_LOC):
                nc.tensor.matmul(yps[:], wo_sb[:, fc, bass.ts(eb, P)],
                                 aon[:, fc, bass.ts(st, STILE)],
                                 start=(fc == 0), stop=(fc == NH_LOC - 1))
            if (eb * N_STILE + st) % 2 == 0:
                nc.scalar.copy(y_sb[:, bass.ts(st, STILE)], yps[:])
            else:
                nc.vector.tensor_copy(y_sb[:, bass.ts(st, STILE)], yps[:])
            if eb >= NJB - 2:
                # stream the last rows out per-stile to shorten the tail
                nc.sync.dma_start(yT_v[eb][:, bass.ts(st, STILE)],
                                  y_sb[:, bass.ts(st, STILE)])
            elif st == N_STILE - 1:
                nc.sync.dma_start(yT_v[eb], y_sb[:])

        with ExitStack() as ph2:
            pp = ph2.enter_context(tc.tile_pool(name="pp", bufs=8))
            sap = ph2.enter_context(tc.tile_pool(name="sap", bufs=2))
            lp = ph2.enter_context(tc.tile_pool(name="lp", bufs=2))
            rlp = ph2.enter_context(tc.tile_pool(name="rlp", bufs=2))
            psst = ph2.enter_context(tc.tile_pool(name="psst", bufs=3, space="PSUM"))
            psao = ph2.enter_context(tc.tile_pool(name="psao", bufs=4, space="PSUM"))
            psrl = ph2.enter_context(tc.tile_pool(name="psrl", bufs=1, space="PSUM"))

            def normalize(pend):
                """Emit the deferred softmax-normalize for one (h, it) tile.
                Runs one tile late so the DVE->ACT->DVE chain never blocks
                PE's in-order queue."""
                h, isl, sacc, ao_ps = pend
                # l[i] = sum_j sacc: 4 single-column matmuls -> [i%128, 4].
                # lt borrows the first 4 columns of the rlb bank (disjoint
                # lifetime: Ln reads lt before the rlb matmul overwrites it).
                rlb = psrl.tile([P, STILE], F32, tag="rlb")
                lt = rlb[:, 0:4]
                for c2 in range(4):
                    nc.tensor.matmul(lt[:, c2 : c2 + 1],
                                     sacc[:, bass.ts(c2, P)], onesc_b[:],
                                     start=True, stop=True)
                # 1/l = exp(-ln(l)) on ACT, cheap at [128, 4]
                lnl = lp.tile([P, 4], F32, tag="lnl")
                nc.scalar.activation(lnl[:], lt[:], AF.Ln)
                rl2 = lp.tile([P, 4], F32, tag="rl2")
                nc.scalar.activation(rl2[:], lnl[:], AF.Exp, scale=-1.0)
                # transpose+broadcast 1/l to [c, i]: mask into eye columns,
                # then ones.T @ masked sums the single nonzero per column
                mskd = rlp.tile([P, STILE], BF16, tag="mskd")
                for c2 in range(4):
                    nc.vector.tensor_scalar_mul(mskd[:, bass.ts(c2, P)],
                                                eye_sb[:], rl2[:, c2 : c2 + 1])
                nc.tensor.matmul(rlb[:], onesf_sb[:], mskd[:],
                                 start=True, stop=True)
                rlb_sb = rlp.tile([P, STILE], F32, tag="rlbs")
                nc.vector.tensor_copy(rlb_sb[:], rlb[:])
                nc.vector.tensor_mul(aon[:, h, isl], ao_ps[:], rlb_sb[:])

            pending = None
            for h in range(NH_LOC):
                for it in range(N_STILE):
                    isl = bass.ts(it, STILE)
                    njb = 4 * it + 4
                    ao_ps = psao.tile([P, STILE], F32, tag="ao")
                    # P-tile running sum over j-blocks (for the softmax denom)
                    sacc = sap.tile([P, STILE], BF16, tag="sacc")
                    # ascending jb: group head (jb=0) is always full-width, so
                    # later diagonal blocks can write narrowed column ranges.
                    for idx, jb in enumerate(range(njb)):
                        t = jb - 4 * it
                        lo = P * t if t > 0 else 0  # masked-out prefix columns
                        csl = slice(lo, STILE)
                        i0 = it * STILE + lo
                        st_ps = psst.tile([P, STILE], F32, tag="st")
                        nc.tensor.matmul(st_ps[:, csl], khat[:, bass.ts(jb, P)],
                                         qhat[:, h, bass.ds(i0, STILE - lo)],
                                         start=True, stop=True)
                        pt = pp.tile([P, STILE], BF16, tag="p")
                        nc.scalar.activation(pt[:, csl], st_ps[:, csl], AF.Exp,
                                             scale=scale_k[:, jb : jb + 1])
                        if t >= 0:
                            nc.vector.tensor_mul(pt[:, csl], pt[:, csl],
                                                 mask_sb[:, t, csl])
                        nc.tensor.matmul(ao_ps[:, csl], vsb[:, jb, :], pt[:, csl],
                                         start=(idx == 0), stop=(idx == njb - 1))
                        if idx == 0:
                            nc.vector.tensor_copy(sacc[:], pt[:])
                        else:
                            nc.vector.tensor_add(sacc[:, csl], sacc[:, csl],
                                                 pt[:, csl])
                    if pending is not None:
                        normalize(pending)
                    pending = (h, isl, sacc, ao_ps)

            # first out-proj groups act as PE filler while the last tile's
            # normalize chain (DVE/ACT) completes
            for eb in range(6):
                for st in range(N_STILE - 1):
                    outproj_group(eb, st, psst)
            normalize(pending)
            for eb in range(6):
                outproj_group(eb, N_STILE - 1, psst)

        # ---------------- Phase 3: out-projection (rest) ----------------
        # bias bo is added host-side after the partial gather
        with ExitStack() as ph3:
            psy = ph3.enter_context(tc.tile_pool(name="psy", bufs=6, space="PSUM"))
            for eb in range(6, NJB):
                for st in range(N_STILE):
                    outproj_group(eb, st, psy)

    _split_excess_waits(nc)
    return nc


_PERM = np.concatenate([np.arange(0, DK, 2), np.arange(1, DK, 2)])  # de-interleave


def _prep_inputs(x, Wq, bq, Wk, bk, Wv, bv, Wo, bo, q_norm_w, k_norm_w):
    """Build the 8 per-core input maps. Core c -> (b = c // 4, g = c % 4)."""
    def bf(a):
        return np.ascontiguousarray(a).astype(_BF)

    wq_p = q_norm_w[_PERM].astype(np.float32)
    wk_p = k_norm_w[_PERM].astype(np.float32)
    with np.errstate(divide="ignore"):
        winvq = np.where(wq_p != 0, 1.0 / np.maximum(wq_p * wq_p, 1e-30), 0.0)
        winvk = np.where(wk_p != 0, 1.0 / np.maximum(wk_p * wk_p, 1e-30), 0.0)

    inv_freq = 1.0 / (10000.0 ** (np.arange(0, DK, 2, dtype=np.float32) / np.float32(DK)))
    freqs = np.arange(S, dtype=np.float32)[:, None] * inv_freq[None, :]
    cosT = np.cos(freqs).T.astype(np.float32)  # [64, S]
    sinT = np.sin(freqs).T.astype(np.float32)
    taba = bf(np.vstack([cosT, cosT]))             # [128, S]
    # tb is computed by cross-half reads (tb_lo = qf_hi*tabb_hi), so the
    # sign rows are pre-swapped: tabb[p] multiplies qf[(p+64)%128]
    tabb = bf(np.vstack([sinT, -sinT]))

    pj = np.arange(P)[:, None, None]
    tt = np.arange(NH_LOC)[None, :, None]
    fi = np.arange(STILE)[None, None, :]
    maskt = ((P * tt + pj) <= fi).astype(np.float32)  # [128, 4, 512]

    xt4_b = []
    for b in range(2):
        xt = x[b].T.astype(np.float32)  # [d, s]
        xt4_b.append(bf(xt.reshape(NC_CHUNKS, P, N_STILE, STILE).transpose(2, 0, 1, 3)))

    in_maps = []
    for core in range(8):
        b, g = divmod(core, NH_LOC)
        hsl = slice(g * NH_LOC * DK, (g + 1) * NH_LOC * DK)
        ksl = slice(g * DK, (g + 1) * DK)

        wq_blk = Wq[hsl].astype(np.float32).copy()  # [512, d]
        # per-head de-interleave permutation + fold q_norm_w
        wq_blk = wq_blk.reshape(NH_LOC, DK, D)[:, _PERM, :] * wq_p[None, :, None]
        wq_t = wq_blk.reshape(NH_LOC * DK, D).T.reshape(NC_CHUNKS, P, NH_LOC * DK)

        wk_blk = Wk[ksl].astype(np.float32)[_PERM, :] * wk_p[:, None]
        wk_t = wk_blk.T.reshape(NC_CHUNKS, P, DK)
        wv_t = Wv[ksl].astype(np.float32).T.reshape(NC_CHUNKS, P, DK)
        wo_t = Wo[:, hsl].astype(np.float32).T.reshape(NH_LOC, P, D)

        bq_blk = bq[hsl].astype(np.float32).reshape(NH_LOC, DK)[:, _PERM].T.copy()  # [128, 4]
        bk_blk = bk[ksl].astype(np.float32)[_PERM][:, None].copy()

        in_maps.append({
            "xt4": xt4_b[b],
            "wq": bf(wq_t), "wk": bf(wk_t), "wv": bf(wv_t), "wo": bf(wo_t),
            "winvq": bf(winvq[:, None]), "winvk": bf(winvk[:, None]),
            "taba": taba, "tabb": tabb,
            "maskt": bf(maskt),
            "bq": np.ascontiguousarray(bq_blk), "bk": bk_blk,
            "eyer": bf(np.eye(P, dtype=np.float32)),
            "onesf": bf(np.ones((P, P), np.float32)),
        })
    return in_maps


_CACHED = {}


def _get_program():
    if "nc" not in _CACHED:
        _CACHED["nc"] = _build_program()
    return _CACHED["nc"]


def kernel(x, Wq, bq, Wk, bk, Wv, bv, Wo, bo, q_norm_w, k_norm_w, _trace=False, _tmpdir=None):
    x = np.asarray(x, np.float32)
    args = [np.asarray(a, np.float32) for a in
            (Wq, bq, Wk, bk, Wv, bv, Wo, bo, q_norm_w, k_norm_w)]
    Wq, bq, Wk, bk, Wv, bv, Wo, bo, q_norm_w, k_norm_w = args

    nc = _get_program()
    in_maps = _prep_inputs(x, Wq, bq, Wk, bk, Wv, bv, Wo, bo, q_norm_w, k_norm_w)
    res = run_bass_kernel_spmd(nc, in_maps, list(range(8)), trace=_trace, tmpdir=_tmpdir)

    out = np.zeros((2, S, D), np.float32)
    for core in range(8):
        b = core // 4
        out[b] += res.results[core]["yT"].astype(np.float32).T
    out += bo[None, None, :]
    # v-bias enters only via softmax-weighted average (weights sum to 1):
    if np.any(bv):
        out += (np.repeat(bv.reshape(4, DK), 4, axis=0).reshape(D) @ Wo.T)[None, None, :]
    kernel._last_result = res
    return out



# revision 68
# speedup vs baseline: 1.0056x; 1.0026x over previous
"""TRN2 Bass kernel for GQA MultiHeadAttention (B=2, S=2048, D=2048, 16 q-heads,
4 kv-heads, d_k=128) with QK-RMSNorm + interleaved RoPE + causal softmax + out-proj.

Sharding: 8 cores = (batch b in {0,1}) x (kv-head group g in {0..3}).
Each core computes its 4 q-heads' attention for its batch and a partial
out-projection y.T = Wo_g @ attn_out_g.T  [2048(e) x 2048(s)].
Host sums the 4 partials per batch and transposes.

Device layouts (all "head-dim on partitions", so no on-device transposes):
  xT   [d=16x128, s]        (moving operand of all projections)
  qT/kT [c=128, s]          RoPE'd + normalized, bf16
  V    [s-in-block=128, 16 blocks, c=128]
  scores ST [j=128, i<=512] via matmul(lhsT=kT-block, rhs=qT-tile)
  P = exp(ST * c0 * rrk[j]) masked; AO.T [c, i] = sum_j V.T P
No softmax max-subtraction: RMSNorm bounds |score| <= sqrt(128), exp is safe.
RoPE pair-interleave is folded into a host-side row permutation of Wq/Wk
(dot products are invariant; V/Wo untouched); the half-swap multiplies read
qf cross-half against a host-pre-swapped sin table (DVE inputs share a base
partition). All reciprocals/rsqrts are ACT exp(-c*ln(x)) from one act table.
Softmax denominators: P tiles are summed over j-blocks on DVE (sacc), then
4 single-column matmuls give l in [i%128, 4]; 1/l is eye-masked and
broadcast back to [c, i] with one ones.T@masked matmul. All matmuls fed by
multi-engine latency chains (pb, lt/rlb) are deferred by one output/tile so
PE's in-order queue never stalls on them; the first out-proj groups act as
PE filler while the last attention tile normalizes. Output yT is bf16;
bo/bv enter host-side after the partial gather.
"""
import sys
import numpy as np
import ml_dtypes

sys.path.insert(0, "/opt/trn_rl_repo")

import concourse.bass as bass  # noqa: E402
import concourse.tile as tile  # noqa: E402
from concourse import mybir  # noqa: E402
from concourse.bass_utils import run_bass_kernel_spmd  # noqa: E402

F32 = mybir.dt.float32
F32R = mybir.dt.float32r
BF16 = mybir.dt.bfloat16
AF = mybir.ActivationFunctionType

P = 128
S = 2048
D = 2048
DK = 128
NH_LOC = 4          # q heads per core
NC_CHUNKS = D // P  # 16 contraction chunks
N_STILE = 4         # s-tiles of 512
STILE = 512
NJB = S // P        # 16 j/s blocks of 128
EPS = 1e-8
C0 = 1.0 / np.sqrt(DK)

_BF = ml_dtypes.bfloat16


_NO_SPLIT_OPCODES = {"UnconditionalBranch", "Call", "RegisterMove", "EventSemaphore"}
_WAIT_LIMIT = {}  # hw instruction structs take a single sync wait


def _split_excess_waits(nc):
    """Walrus codegen allows only 1-2 sync waits per instruction struct; Tile
    can emit more. Move excess waits onto same-engine NoOps inserted before."""
    import bass_rust
    counter = [0]
    for fn in nc.m.functions:
        for blk in fn.blocks:
            out = []
            changed = False
            for inst in blk.instructions:
                si = inst.sync_info
                limit = _WAIT_LIMIT.get(inst.opcode, 1)
                if (si is not None and len(si.on_wait) > limit
                        and inst.opcode not in _NO_SPLIT_OPCODES):
                    waits = list(si.on_wait)
                    for w in waits[:-limit]:
                        counter[0] += 1
                        nop = bass_rust.InstNoOp(
                            name=f"I-wsplit-{counter[0]}", ins=[], outs=[])
                        nop.engine = inst.engine
                        nop.sync_info = mybir.SyncInfo(on_wait=[w], on_update=[])
                        out.append(nop)
                    inst.sync_info = mybir.SyncInfo(
                        on_wait=waits[-limit:], on_update=list(si.on_update))
                    changed = True
                out.append(inst)
            if changed:
                blk.instructions = out
    return counter[0]


def _build_program():
    nc = bass.Bass()

    ext = {}

    def inp(name, shape, dt):
        ext[name] = nc.declare_dram_parameter(name, list(shape), dt, isOutput=False)
        return ext[name]

    xt4 = inp("xt4", (N_STILE, NC_CHUNKS, P, STILE), BF16)
    wq = inp("wq", (NC_CHUNKS, P, NH_LOC * DK), BF16)
    wk = inp("wk", (NC_CHUNKS, P, DK), BF16)
    wv = inp("wv", (NC_CHUNKS, P, DK), BF16)
    wo = inp("wo", (NH_LOC, P, D), BF16)
    winvq = inp("winvq", (P, 1), BF16)
    winvk = inp("winvk", (P, 1), BF16)
    taba = inp("taba", (P, S), BF16)  # [cos; cos]
    tabb = inp("tabb", (P, S), BF16)  # [-sin; sin]
    maskt = inp("maskt", (P, NH_LOC, STILE), BF16)
    bq = inp("bq", (P, NH_LOC), F32)
    bk = inp("bk", (P, 1), F32)
    eyer = inp("eyer", (P, P), BF16)
    onesf = inp("onesf", (P, P), BF16)
    yT = nc.declare_dram_parameter("yT", [D, S], BF16, isOutput=True)

    from contextlib import ExitStack

    with tile.TileContext(nc) as tc, ExitStack() as top:
        const = top.enter_context(tc.tile_pool(name="const", bufs=1))

        wq_sb = const.tile([P, NC_CHUNKS, NH_LOC * DK], BF16, tag="wq")
        wk_sb = const.tile([P, NC_CHUNKS, DK], BF16, tag="wk")
        wv_sb = const.tile([P, NC_CHUNKS, DK], BF16, tag="wv")
        winvq_sb = const.tile([P, 1], BF16, tag="winvq")
        winvk_sb = const.tile([P, 1], BF16, tag="winvk")
        taba_sb = const.tile([P, S], BF16, tag="taba")
        tabb_sb = const.tile([P, S], BF16, tag="tabb")
        mask_sb = const.tile([P, NH_LOC, STILE], BF16, tag="mask")
        bq_sb = const.tile([P, NH_LOC], F32, tag="bq")
        bk_sb = const.tile([P, 1], F32, tag="bk")
        wo_sb = const.tile([P, NH_LOC, D], BF16, tag="wo")

        eye_sb = const.tile([P, P], BF16, tag="eye")
        nc.sync.dma_start(eye_sb[:], eyer[:])
        onesf_sb = const.tile([P, P], BF16, tag="onesf")
        nc.sync.dma_start(onesf_sb[:], onesf[:])
        onesc_b = const.tile([P, 1], BF16, tag="onesc")
        nc.vector.memset(onesc_b[:], 1.0)
        eps_sb = const.tile([P, 1], F32, tag="eps")
        nc.vector.memset(eps_sb[:], EPS)
        lnc0_sb = const.tile([P, 1], F32, tag="lnc0")
        nc.vector.memset(lnc0_sb[:], float(np.log(C0)))

        # persistent activation tensors
        qhat = const.tile([P, NH_LOC, S], BF16, tag="qhat")   # [c, h, s]
        khat = const.tile([P, S], BF16, tag="khat")           # [c, s]
        vsb = const.tile([P, NJB, DK], BF16, tag="v")         # [s%128, block, c]
        aon = const.tile([P, NH_LOC, S], BF16, tag="aon")     # [c, h, i]
        scale_k = const.tile([P, NJB], F32, tag="scale_k")    # c0 * rrk, [j%128, block]

        # ------- Phase 1: projections + RMS + RoPE, fused per (output, s-tile) -------
        with ExitStack() as ph1:
            xp = ph1.enter_context(tc.tile_pool(name="xp", bufs=2))
            t1p = ph1.enter_context(tc.tile_pool(name="t1p", bufs=4))
            rp = ph1.enter_context(tc.tile_pool(name="rp", bufs=3))
            ps1 = ph1.enter_context(tc.tile_pool(name="ps1", bufs=3, space="PSUM"))
            pskl = ph1.enter_context(tc.tile_pool(name="pskl", bufs=1, space="PSUM"))
            psb = ph1.enter_context(tc.tile_pool(name="psb", bufs=2, space="PSUM"))

            ps_kl = pskl.tile([P, NJB], F32, tag="pskl")

            # startup-critical loads first: q weights on SP, first x tile on
            # the ACT queue — parallel dispatch halves time-to-first-matmul
            xt0 = xp.tile([P, NC_CHUNKS, STILE], BF16, tag="xt", name="xt0")
            nc.sync.dma_start(wq_sb[:, 0:1, :], wq[0:1].rearrange("c p m -> p c m"))
            nc.scalar.dma_start(xt0[:, 0:1, :], xt4[0, 0:1].rearrange("c p s -> p c s"))
            for g4 in range(0, NC_CHUNKS, 4):
                sl4 = slice(max(g4, 1), g4 + 4)
                nc.sync.dma_start(wq_sb[:, sl4, :], wq[sl4].rearrange("c p m -> p c m"))
                nc.sync.dma_start(xt0[:, sl4, :],
                                  xt4[0, sl4].rearrange("c p s -> p c s"))
            nc.sync.dma_start(bq_sb[:], bq[:])
            nc.sync.dma_start(winvq_sb[:], winvq[:])
            nc.sync.dma_start(taba_sb[:], taba[:])
            nc.sync.dma_start(tabb_sb[:], tabb[:])
            for g4 in range(0, NC_CHUNKS, 4):
                sl4 = slice(g4, g4 + 4)
                nc.sync.dma_start(wk_sb[:, sl4, :], wk[sl4].rearrange("c p m -> p c m"))
                nc.sync.dma_start(wv_sb[:, sl4, :], wv[sl4].rearrange("c p m -> p c m"))
            nc.sync.dma_start(winvk_sb[:], winvk[:])
            nc.sync.dma_start(bk_sb[:], bk[:])
            # mask is needed by the very first phase-2 tile; load it early
            nc.sync.dma_start(mask_sb[:], maskt[:])

            # deferred q-rms apply: pb broadcast matmul + qhat mul emitted one
            # output later, so the Ln/Exp latency hides under the next
            # output's projection matmuls instead of stalling PE in-order
            pend_q = [None]

            def flush_q():
                if pend_q[0] is None:
                    return
                oi, rt, mskq, pb, ssl_p = pend_q[0]
                pend_q[0] = None
                nc.tensor.matmul(pb[:], onesf_sb[:], mskq[:],
                                 start=True, stop=True)
                nc.vector.tensor_mul(qhat[:, oi, ssl_p], rt[:], pb[:])

            for st in range(N_STILE):
                if st == 0:
                    xt = xt0
                else:
                    xt = xp.tile([P, NC_CHUNKS, STILE], BF16, tag="xt")
                    for g4 in range(0, NC_CHUNKS, 4):
                        sl4 = slice(g4, g4 + 4)
                        nc.sync.dma_start(xt[:, sl4, :],
                                          xt4[st, sl4].rearrange("c p s -> p c s"))
                ssl = bass.ts(st, STILE)

                # q heads first, then v, then k (k last: its psum banks are
                # freed by the fast ACT reader right before phase 2 reuses them)
                for oi in list(range(NH_LOC)) + ["v", "k"]:
                    if oi == "v":
                        # v: output [s-block=128, c=128], 4 s-blocks per s-tile
                        ptv = ps1.tile([P, STILE], F32, tag="proj", name="ptv")
                        for sb in range(4):
                            for ch in range(NC_CHUNKS):
                                nc.tensor.matmul(ptv[:, bass.ts(sb, DK)],
                                                 xt[:, ch, bass.ts(sb, P)], wv_sb[:, ch, :],
                                                 start=(ch == 0), stop=(ch == NC_CHUNKS - 1))
                        flush_q()
                        for sb in range(4):
                            nc.vector.tensor_copy(vsb[:, st * 4 + sb, :], ptv[:, bass.ts(sb, DK)])
                        continue
                    is_q = oi != "k"
                    pt = ps1.tile([P, STILE], F32, tag="proj")
                    for ch in range(NC_CHUNKS):
                        lw = wq_sb[:, ch, bass.ts(oi, DK)] if is_q else wk_sb[:, ch, :]
                        nc.tensor.matmul(pt[:], lw, xt[:, ch, :],
                                         start=(ch == 0), stop=(ch == NC_CHUNKS - 1))
                    flush_q()
                    bias_ap = bq_sb[:, oi : oi + 1] if is_q else bk_sb[:, 0:1]
                    qf = t1p.tile([P, STILE], BF16, tag="qf")
                    nc.scalar.activation(qf[:], pt[:], AF.Identity, bias=bias_ap)

                    # sumsq for rms (q: [1,s] orientation; k: transposed [s,1])
                    sq = t1p.tile([P, STILE], BF16, tag="sq")
                    nc.vector.tensor_mul(sq[:], qf[:], qf[:])
                    if is_q:
                        # sumsq transposed: 4 single-col matmuls -> [s%128, 4],
                        # parked in the first 4 columns of the pb bank
                        pb = psb.tile([P, STILE], F32, tag="rqb")
                        ltq = pb[:, 0:4]
                        for sb in range(4):
                            nc.tensor.matmul(ltq[:, sb : sb + 1],
                                             sq[:, bass.ts(sb, P)], winvq_sb[:],
                                             start=True, stop=True)
                        # rrq = 1/sqrt(mean+eps) = exp(-0.5*ln(v/DK + eps))
                        lnq = t1p.tile([P, 4], F32, tag="lnv")
                        nc.scalar.activation(lnq[:], ltq[:], AF.Ln,
                                             bias=eps_sb[:], scale=1.0 / DK)
                        rrq2 = t1p.tile([P, 4], F32, tag="rrq")
                        nc.scalar.activation(rrq2[:], lnq[:], AF.Exp, scale=-0.5)
                        mskq = rp.tile([P, STILE], BF16, tag="mskq")
                        for c2 in range(4):
                            nc.vector.tensor_scalar_mul(mskq[:, bass.ts(c2, P)],
                                                        eye_sb[:],
                                                        rrq2[:, c2 : c2 + 1])
                    else:
                        for sb in range(4):
                            nc.tensor.matmul(ps_kl[:, st * 4 + sb : st * 4 + sb + 1],
                                             sq[:, bass.ts(sb, P)], winvk_sb[:],
                                             start=True, stop=True)

                    # RoPE on de-interleaved halves:
                    #   rt = qf*[cos;cos] + swap_halves(qf)*[-sin;sin]
                    # (swap done by cross-partition-offset reads of qf)
                    ta = rp.tile([P, STILE], BF16, tag="ta")
                    tb = rp.tile([P, STILE], BF16, tag="tb")
                    nc.vector.tensor_mul(ta[:], qf[:], taba_sb[:, ssl])
                    nc.vector.tensor_mul(tb[0:64, :], qf[64:P, :], tabb_sb[64:P, ssl])
                    nc.vector.tensor_mul(tb[64:P, :], qf[0:64, :], tabb_sb[0:64, ssl])
                    if is_q:
                        rt = rp.tile([P, STILE], BF16, tag="rope")
                        nc.vector.tensor_add(rt[:], ta[:], tb[:])
                        pend_q[0] = (oi, rt, mskq, pb, ssl)
                    else:
                        nc.vector.tensor_add(khat[:, ssl], ta[:], tb[:])

                # k-side scale for this s-tile: c0 / rms_k as [j%128, block]
                # = exp(-0.5*ln(mean+eps) + ln(c0)), single-table Ln/Exp
                ksl4 = bass.ts(st, 4)
                lnk = t1p.tile([P, 4], F32, tag="lnk")
                nc.scalar.activation(lnk[:], ps_kl[:, ksl4], AF.Ln,
                                     bias=eps_sb[:], scale=1.0 / DK)
                nc.scalar.activation(scale_k[:, ksl4], lnk[:], AF.Exp,
                                     scale=-0.5, bias=lnc0_sb[:])




        # ---------------- Phase 2: attention ----------------
        nc.sync.dma_start(wo_sb[:], wo.rearrange("f p e -> p f e"))

        yp = top.enter_context(tc.tile_pool(name="yp", bufs=6))
        yT_v = yT.rearrange("(eb p) s -> eb p s", p=P)
        y_sbs = {}

        def outproj_group(eb, st, pspool):
            """One (eb, st) out-projection accumulation + PSUM drain."""
            if eb not in y_sbs:
                y_sbs[eb] = yp.tile([P, S], BF16, tag="ysb", name=f"ysb{eb}")
            y_sb = y_sbs[eb]
            yps = pspool.tile([P, STILE], F32, tag="st", name=f"yps{eb}_{st}")
            for fc in range(NH_LOC):
                nc.tensor.matmul(yps[:], wo_sb[:, fc, bass.ts(eb, P)],
                                 aon[:, fc, bass.ts(st, STILE)],
                                 start=(fc == 0), stop=(fc == NH_LOC - 1))
            if (eb * N_STILE + st) % 2 == 0:
                nc.scalar.copy(y_sb[:, bass.ts(st, STILE)], yps[:])
            else:
                nc.vector.tensor_copy(y_sb[:, bass.ts(st, STILE)], yps[:])
            if eb >= NJB - 2:
                # stream the last rows out per-stile to shorten the tail
                nc.sync.dma_start(yT_v[eb][:, bass.ts(st, STILE)],
                                  y_sb[:, bass.ts(st, STILE)])
            elif st == N_STILE - 1:
                nc.sync.dma_start(yT_v[eb], y_sb[:])

        with ExitStack() as ph2:
            pp = ph2.enter_context(tc.tile_pool(name="pp", bufs=8))
            sap = ph2.enter_context(tc.tile_pool(name="sap", bufs=2))
            lp = ph2.enter_context(tc.tile_pool(name="lp", bufs=2))
            rlp = ph2.enter_context(tc.tile_pool(name="rlp", bufs=2))
            psst = ph2.enter_context(tc.tile_pool(name="psst", bufs=3, space="PSUM"))
            psao = ph2.enter_context(tc.tile_pool(name="psao", bufs=4, space="PSUM"))
            psrl = ph2.enter_context(tc.tile_pool(name="psrl", bufs=1, space="PSUM"))

            def normalize(pend):
                """Emit the deferred softmax-normalize for one (h, it) tile.
                Runs one tile late so the DVE->ACT->DVE chain never blocks
                PE's in-order queue."""
                h, isl, sacc, ao_ps = pend
                # l[i] = sum_j sacc: 4 single-column matmuls -> [i%128, 4].
                # lt borrows the first 4 columns of the rlb bank (disjoint
                # lifetime: Ln reads lt before the rlb matmul overwrites it).
                rlb = psrl.tile([P, STILE], F32, tag="rlb")
                lt = rlb[:, 0:4]
                for c2 in range(4):
                    nc.tensor.matmul(lt[:, c2 : c2 + 1],
                                     sacc[:, bass.ts(c2, P)], onesc_b[:],
                                     start=True, stop=True)
                # 1/l = exp(-ln(l)) on ACT, cheap at [128, 4]
                lnl = lp.tile([P, 4], F32, tag="lnl")
                nc.scalar.activation(lnl[:], lt[:], AF.Ln)
                rl2 = lp.tile([P, 4], F32, tag="rl2")
                nc.scalar.activation(rl2[:], lnl[:], AF.Exp, scale=-1.0)
                # transpose+broadcast 1/l to [c, i]: mask into eye columns,
                # then ones.T @ masked sums the single nonzero per column
                mskd = rlp.tile([P, STILE], BF16, tag="mskd")
                for c2 in range(4):
                    nc.vector.tensor_scalar_mul(mskd[:, bass.ts(c2, P)],
                                                eye_sb[:], rl2[:, c2 : c2 + 1])
                nc.tensor.matmul(rlb[:], onesf_sb[:], mskd[:],
                                 start=True, stop=True)
                rlb_sb = rlp.tile([P, STILE], F32, tag="rlbs")
                nc.vector.tensor_copy(rlb_sb[:], rlb[:])
                nc.vector.tensor_mul(aon[:, h, isl], ao_ps[:], rlb_sb[:])

            pending = None
            for h in range(NH_LOC):
                for it in range(N_STILE):
                    isl = bass.ts(it, STILE)
                    njb = 4 * it + 4
                    ao_ps = psao.tile([P, STILE], F32, tag="ao")
                    # P-tile running sum over j-blocks (for the softmax denom)
                    sacc = sap.tile([P, STILE], BF16, tag="sacc")
                    # ascending jb: group head (jb=0) is always full-width, so
                    # later diagonal blocks can write narrowed column ranges.
                    for idx, jb in enumerate(range(njb)):
                        t = jb - 4 * it
                        lo = P * t if t > 0 else 0  # masked-out prefix columns
                        csl = slice(lo, STILE)
                        i0 = it * STILE + lo
                        st_ps = psst.tile([P, STILE], F32, tag="st")
                        nc.tensor.matmul(st_ps[:, csl], khat[:, bass.ts(jb, P)],
                                         qhat[:, h, bass.ds(i0, STILE - lo)],
                                         start=True, stop=True)
                        pt = pp.tile([P, STILE], BF16, tag="p")
                        nc.scalar.activation(pt[:, csl], st_ps[:, csl], AF.Exp,
                                             scale=scale_k[:, jb : jb + 1])
                        if t >= 0:
                            nc.vector.tensor_mul(pt[:, csl], pt[:, csl],
                                                 mask_sb[:, t, csl])
                        nc.tensor.matmul(ao_ps[:, csl], vsb[:, jb, :], pt[:, csl],
                                         start=(idx == 0), stop=(idx == njb - 1))
                        if idx == 0:
                            nc.vector.tensor_copy(sacc[:], pt[:])
                        else:
                            nc.vector.tensor_add(sacc[:, csl], sacc[:, csl],
                                                 pt[:, csl])
                    if pending is not None:
                        normalize(pending)
                    pending = (h, isl, sacc, ao_ps)

            # first out-proj groups act as PE filler while the last tile's
            # normalize chain (DVE/ACT) completes
            for eb in range(6):
                for st in range(N_STILE - 1):
                    outproj_group(eb, st, psst)
            normalize(pending)
            for eb in range(6):
                outproj_group(eb, N_STILE - 1, psst)

        # ---------------- Phase 3: out-projection (rest) ----------------
        # bias bo is added host-side after the partial gather
        with ExitStack() as ph3:
            psy = ph3.enter_context(tc.tile_pool(name="psy", bufs=6, space="PSUM"))
            for eb in range(6, NJB):
                for st in range(N_STILE):
                    outproj_group(eb, st, psy)

    _split_excess_waits(nc)
    return nc


_PERM = np.concatenate([np.arange(0, DK, 2), np.arange(1, DK, 2)])  # de-interleave


def _prep_inputs(x, Wq, bq, Wk, bk, Wv, bv, Wo, bo, q_norm_w, k_norm_w):
    """Build the 8 per-core input maps. Core c -> (b = c // 4, g = c % 4)."""
    def bf(a):
        return np.ascontiguousarray(a).astype(_BF)

    wq_p = q_norm_w[_PERM].astype(np.float32)
    wk_p = k_norm_w[_PERM].astype(np.float32)
    with np.errstate(divide="ignore"):
        winvq = np.where(wq_p != 0, 1.0 / np.maximum(wq_p * wq_p, 1e-30), 0.0)
        winvk = np.where(wk_p != 0, 1.0 / np.maximum(wk_p * wk_p, 1e-30), 0.0)

    inv_freq = 1.0 / (10000.0 ** (np.arange(0, DK, 2, dtype=np.float32) / np.float32(DK)))
    freqs = np.arange(S, dtype=np.float32)[:, None] * inv_freq[None, :]
    cosT = np.cos(freqs).T.astype(np.float32)  # [64, S]
    sinT = np.sin(freqs).T.astype(np.float32)
    taba = bf(np.vstack([cosT, cosT]))             # [128, S]
    # tb is computed by cross-half reads (tb_lo = qf_hi*tabb_hi), so the
    # sign rows are pre-swapped: tabb[p] multiplies qf[(p+64)%128]
    tabb = bf(np.vstack([sinT, -sinT]))

    pj = np.arange(P)[:, None, None]
    tt = np.arange(NH_LOC)[None, :, None]
    fi = np.arange(STILE)[None, None, :]
    maskt = ((P * tt + pj) <= fi).astype(np.float32)  # [128, 4, 512]

    xt4_b = []
    for b in range(2):
        xt = x[b].T.astype(np.float32)  # [d, s]
        xt4_b.append(bf(xt.reshape(NC_CHUNKS, P, N_STILE, STILE).transpose(2, 0, 1, 3)))

    in_maps = []
    for core in range(8):
        b, g = divmod(core, NH_LOC)
        hsl = slice(g * NH_LOC * DK, (g + 1) * NH_LOC * DK)
        ksl = slice(g * DK, (g + 1) * DK)

        wq_blk = Wq[hsl].astype(np.float32).copy()  # [512, d]
        # per-head de-interleave permutation + fold q_norm_w
        wq_blk = wq_blk.reshape(NH_LOC, DK, D)[:, _PERM, :] * wq_p[None, :, None]
        wq_t = wq_blk.reshape(NH_LOC * DK, D).T.reshape(NC_CHUNKS, P, NH_LOC * DK)

        wk_blk = Wk[ksl].astype(np.float32)[_PERM, :] * wk_p[:, None]
        wk_t = wk_blk.T.reshape(NC_CHUNKS, P, DK)
        wv_t = Wv[ksl].astype(np.float32).T.reshape(NC_CHUNKS, P, DK)
        wo_t = Wo[:, hsl].astype(np.float32).T.reshape(NH_LOC, P, D)

        bq_blk = bq[hsl].astype(np.float32).reshape(NH_LOC, DK)[:, _PERM].T.copy()  # [128, 4]
        bk_blk = bk[ksl].astype(np.float32)[_PERM][:, None].copy()

        in_maps.append({
            "xt4": xt4_b[b],
            "wq": bf(wq_t), "wk": bf(wk_t), "wv": bf(wv_t), "wo": bf(wo_t),
            "winvq": bf(winvq[:, None]), "winvk": bf(winvk[:, None]),
            "taba": taba, "tabb": tabb,
            "maskt": bf(maskt),
            "bq": np.ascontiguousarray(bq_blk), "bk": bk_blk,
            "eyer": bf(np.eye(P, dtype=np.float32)),
            "onesf": bf(np.ones((P, P), np.float32)),
        })
    return in_maps


_CACHED = {}


def _get_program():
    if "nc" not in _CACHED:
        _CACHED["nc"] = _build_program()
    return _CACHED["nc"]


def kernel(x, Wq, bq, Wk, bk, Wv, bv, Wo, bo, q_norm_w, k_norm_w, _trace=False, _tmpdir=None):
    x = np.asarray(x, np.float32)
    args = [np.asarray(a, np.float32) for a in
            (Wq, bq, Wk, bk, Wv, bv, Wo, bo, q_norm_w, k_norm_w)]
    Wq, bq, Wk, bk, Wv, bv, Wo, bo, q_norm_w, k_norm_w = args

    nc = _get_program()
    in_maps = _prep_inputs(x, Wq, bq, Wk, bk, Wv, bv, Wo, bo, q_norm_w, k_norm_w)
    res = run_bass_kernel_spmd(nc, in_maps, list(range(8)), trace=_trace, tmpdir=_tmpdir)

    out = np.zeros((2, S, D), np.float32)
    for core in range(8):
        b = core // 4
        out[b] += res.results[core]["yT"].astype(np.float32).T
    out += bo[None, None, :]
    # v-bias enters only via softmax-weighted average (weights sum to 1):
    if np.any(bv):
        out += (np.repeat(bv.reshape(4, DK), 4, axis=0).reshape(D) @ Wo.T)[None, None, :]
    kernel._last_result = res
    return out

